# revision 1
# baseline (speedup 1.0000x reference)
"""Trainium2 Bass kernel for nn_DocREModel_Triangle (DocRE block-bilinear model).

Strategy (8 NeuronCores, single SPMD NEFF):
  Phase 1 (pair-parallel): core c owns batch b=c//4 and entity rows
  i in [6*(c%4), 6*(c%4)+6) -> 144 (i,j) pairs. Computes, all in a
  transposed layout (feature dim on partitions, pairs on the free dim):
  mention gather + logsumexp entity embeddings, attention-row gather +
  mention-sum (fused into a selector matmul), pairwise head-products +
  l-normalization, context vectors rs^T, and the two tanh extractors
  hs^T/ts^T [768, 144] (bf16).
  Collectives: AllToAll redistributes hs^T by s-slices (hs rows are
  emitted in (s, k) order via a host-side column permutation of Wh so
  slices are contiguous); AllGather replicates ts^T.
  Phase 2 (contraction-parallel): core c holds Wp rows (k, s in
  [8c,8c+8), t) resident in SBUF (bf16, 9.4 MB) and accumulates
  feature^T = sum over its 6144 (k,s,t) rows of
  hs[k,s]*ts[k,t] * Wp[(k,s,t), :] for all 1152 pairs, then the
  classifier; an AllReduce sums the logits partials [97, 1152].
"""

import numpy as np
import ml_dtypes

bf16 = ml_dtypes.bfloat16

B, L, H, NH = 2, 512, 768, 12
NE, NM = 24, 4
E, BS, C = 768, 64, 97
K = E // BS                      # 12 blocks
NCORE = 8
IPC = NE // 4                    # 6 i-rows per core (4 cores per batch elem)
PL = IPC * NE                    # 144 local pairs
NP = B * NE * NE                 # 1152 global pairs
SL = 64 // NCORE                 # 8 s-values per core
KST = K * SL * BS                # 6144 contraction rows per core
NKT = KST // 128                 # 48 contraction tiles
NCHUNK = 4                       # pair chunks of 288 (= 2 pair-blocks)
CW = NP // NCHUNK                # 288

# column permutation taking extractor output row e' = s*12+k from old e = k*64+s
_PERM = np.array([k * 64 + s for s in range(64) for k in range(K)])


def _host_prep(inputs):
    """Build the 8 per-core input maps from the full inputs."""
    seq = np.ascontiguousarray(inputs["sequence_output"], dtype=np.float32)
    att = np.ascontiguousarray(inputs["attention"], dtype=np.float32)
    Wh = np.asarray(inputs["Wh"], dtype=np.float32)
    bh = np.asarray(inputs["bh"], dtype=np.float32)
    Wt = np.asarray(inputs["Wt"], dtype=np.float32)
    bt = np.asarray(inputs["bt"], dtype=np.float32)
    Wp = np.asarray(inputs["Wp"], dtype=np.float32)
    Wc = np.asarray(inputs["Wc"], dtype=np.float32)
    bc = np.asarray(inputs["bc"], dtype=np.float32)
    mpos = np.asarray(inputs["mention_pos"]).astype(np.int64)

    wh1p = np.ascontiguousarray(Wh[:H][:, _PERM].astype(bf16))
    wh2p = np.ascontiguousarray(Wh[H:][:, _PERM].astype(bf16))
    wt1 = np.ascontiguousarray(Wt[:H].astype(bf16))
    wt2 = np.ascontiguousarray(Wt[H:].astype(bf16))
    bh_p = np.ascontiguousarray(bh[_PERM].reshape(6, 128).T.astype(np.float32))
    bt_t = np.ascontiguousarray(bt.reshape(6, 128).T.astype(np.float32))
    wc_bf = np.ascontiguousarray(Wc.astype(bf16))
    bc_t = np.ascontiguousarray(bc.reshape(C, 1).astype(np.float32))
    wp4 = Wp.reshape(K, 64, BS, H)

    in_maps = []
    for c in range(NCORE):
        b = c // 4
        i0 = (c % 4) * IPC
        ents = list(range(NE)) + list(range(i0, i0 + IPC))  # 24 j-side + 6 i-side

        # mention gather row indices: 4 m-blocks at 32-partition alignment
        mi = np.zeros((128, 1), dtype=np.int32)
        for m in range(NM):
            for e_i, ent in enumerate(ents):
                mi[m * 32 + e_i, 0] = mpos[b, ent, m]
        ment_idx = np.ascontiguousarray(mi)

        # host-gathered attention rows (sharding by mention positions):
        # [128, NH, L] bf16, rows = 4m x 30ents (+ 8 pad) per head
        ai = np.zeros((128, NH), dtype=np.int32)
        for h in range(NH):
            for m in range(NM):
                for e_i, ent in enumerate(ents):
                    ai[m * 30 + e_i, h] = h * L + mpos[b, ent, m]
        att_gb = np.ascontiguousarray(
            att[b].reshape(NH * L, L)[ai.T].transpose(1, 0, 2).astype(bf16))

        wp_sl = np.ascontiguousarray(
            wp4[:, SL * c : SL * (c + 1)].reshape(KST, H).astype(bf16)
        )

        in_maps.append(
            {
                "seq_bf": np.ascontiguousarray(seq[b].astype(bf16)),
                "seq_f32": seq[b],
                "att_gb": att_gb,
                "ment_idx": ment_idx,
                "wh1p": wh1p,
                "wh2p": wh2p,
                "wt1": wt1,
                "wt2": wt2,
                "bh_p": bh_p,
                "bt_t": bt_t,
                "wp_sl": wp_sl,
                "wc_bf": wc_bf,
                "bc_t": bc_t,
            }
        )
    return in_maps


def _build_consts():
    # selector [120, 30]: sums the 4 mention rows per entity during the
    # attention transpose-matmul (the /4 mean cancels in the normalization)
    S = np.zeros((120, 30), dtype=bf16)
    for m in range(NM):
        for e_i in range(30):
            S[m * 30 + e_i, e_i] = 1.0
    S2 = np.zeros((128, 30), dtype=bf16)
    for m in range(NM):
        for e_i in range(30):
            S2[m * 32 + e_i, e_i] = 1.0
    ones_bf = np.ones((128, 1), dtype=bf16)
    ones_row = np.ones((1, 128), dtype=np.float32)
    sel2 = np.zeros((8, 4, 128), dtype=bf16)
    for sp in range(4):
        sel2[2 * sp, sp, 0:64] = 1
        sel2[2 * sp + 1, sp, 64:128] = 1
    mask = np.ones((C, NP), dtype=np.float32)
    for c in range(NCORE):
        for il in range(IPC):
            ig = (c % 4) * IPC + il
            mask[:, c * PL + il * NE + ig] = 0.0
    return S, S2, ones_bf, ones_row, sel2, mask


class _SubStop(Exception):
    def __init__(self, tile_ap):
        self.tile_ap = tile_ap


def build_bass():
    import os
    import concourse.bass as bass
    import concourse.mybir as mybir
    import concourse.tile as tile
    from concourse.bacc import Bacc

    f32 = mybir.dt.float32
    bft = mybir.dt.bfloat16
    i32 = mybir.dt.int32
    AF = mybir.ActivationFunctionType
    ALU = mybir.AluOpType

    nc = Bacc("TRN2", num_devices=NCORE)

    # ---- I/O ----
    seq_bf = nc.dram_tensor("seq_bf", [L, H], bft, kind="ExternalInput")
    seq_f32 = nc.dram_tensor("seq_f32", [L, H], f32, kind="ExternalInput")
    att_gb = nc.dram_tensor("att_gb", [128, NH, L], bft, kind="ExternalInput")
    ment_idx = nc.dram_tensor("ment_idx", [128, 1], i32, kind="ExternalInput")
    wh1p = nc.dram_tensor("wh1p", [H, E], bft, kind="ExternalInput")
    wh2p = nc.dram_tensor("wh2p", [H, E], bft, kind="ExternalInput")
    wt1 = nc.dram_tensor("wt1", [H, E], bft, kind="ExternalInput")
    wt2 = nc.dram_tensor("wt2", [H, E], bft, kind="ExternalInput")
    bh_p = nc.dram_tensor("bh_p", [128, 6], f32, kind="ExternalInput")
    bt_t = nc.dram_tensor("bt_t", [128, 6], f32, kind="ExternalInput")
    wp_sl = nc.dram_tensor("wp_sl", [KST, H], bft, kind="ExternalInput")
    wc_bf = nc.dram_tensor("wc_bf", [H, C], bft, kind="ExternalInput")
    bc_t = nc.dram_tensor("bc_t", [C, 1], f32, kind="ExternalInput")
    out_lgT = nc.dram_tensor("out_lgT", [C, NP], f32, kind="ExternalOutput")

    S_np, S2_np, ones_np, onesrow_np, sel2_np, mask_np = _build_consts()
    S_dr = nc.inline_tensor(S_np, "sel_const")
    S2_dr = nc.inline_tensor(S2_np, "s2_const")
    ones_dr = nc.inline_tensor(ones_np, "ones_const")
    onesrow_dr = nc.inline_tensor(onesrow_np, "onesrow_const")
    sel2_dr = nc.inline_tensor(sel2_np, "sel2_const")
    mask_dr = nc.inline_tensor(mask_np, "mask_const")

    # collective buffers
    hs_cc_in = nc.dram_tensor("hs_cc_in", [E, PL], bft)
    hs_cc_out = nc.dram_tensor("hs_cc_out", [E, PL], bft)
    ts_cc_in = nc.dram_tensor("ts_cc_in", [E, PL], bft)
    ts_cc_o1 = nc.dram_tensor("ts_cc_o1", [NCORE, E // 2, PL], bft, addr_space="Shared")
    ts_cc_o2 = nc.dram_tensor("ts_cc_o2", [NCORE, E // 2, PL], bft, addr_space="Shared")
    lg_cc_in = nc.dram_tensor("lg_cc_in", [NCHUNK, C, CW], f32)
    lg_cc_out = nc.dram_tensor("lg_cc_out", [NCHUNK, C, CW], f32, addr_space="Shared")
    groups = [list(range(NCORE))]

    with tile.TileContext(nc) as tc:
        with (
            tc.tile_pool(name="gpool", bufs=1) as gpool,
            tc.tile_pool(name="persist", bufs=1) as persist,
        ):
            # ---------- whole-kernel-lifetime weights / constants ----------
            wp_sb = gpool.tile([128, NKT, H], bft)
            wc_sb = gpool.tile([128, 6, C], bft)
            bc_sb = gpool.tile([C, 1], f32)
            nc.gpsimd.dma_start(out=bc_sb, in_=bc_t[:])
            mask_sb = gpool.tile([C, NP], f32)
            ones_sb = gpool.tile([128, 1], bft)
            nc.gpsimd.dma_start(out=ones_sb, in_=ones_dr[:])
            onesrow_sb = gpool.tile([1, 128], f32)
            nc.gpsimd.dma_start(out=onesrow_sb, in_=onesrow_dr[:])
            sel2_sb = gpool.tile([8, 4, 128], bft)
            nc.gpsimd.dma_start(out=sel2_sb, in_=sel2_dr[:])

            hs_sb = persist.tile([128, 6, PL], bft)
            ts_sb = persist.tile([128, 6, PL], bft)
            lg_sb = persist.tile([C, NP], f32)

            with (
                tc.tile_pool(name="p1", bufs=1) as p1,
                tc.tile_pool(name="ps1", bufs=2, space="PSUM") as ps1,
            ):
              try:
                ksub = int(os.environ.get("KSUB", "9"))
                # ---------- critical-path inputs first ----------
                mi_sb = p1.tile([128, 1], i32)
                nc.gpsimd.dma_start(out=mi_sb, in_=ment_idx[:])
                S_sb = p1.tile([120, 30], bft)
                nc.gpsimd.dma_start(out=S_sb, in_=S_dr[:])
                S2_sb = p1.tile([128, 30], bft)
                nc.gpsimd.dma_start(out=S2_sb, in_=S2_dr[:])
                att_b = p1.tile([128, NH, L], bft)
                nc.sync.dma_start(out=att_b, in_=att_gb[:])
                seq_sb = p1.tile([128, 4, H], bft)
                nc.sync.dma_start(out=seq_sb, in_=seq_bf[:].rearrange("(a p) h -> p a h", p=128))
                nc.sync.dma_start(out=wp_sb, in_=wp_sl[:].rearrange("(a p) h -> p a h", p=128))
                nc.sync.dma_start(out=wc_sb, in_=wc_bf[:].rearrange("(a p) c -> p a c", p=128))
                nc.sync.dma_start(out=mask_sb, in_=mask_dr[:])
                # ---------- phase-1 weights ----------
                wh1_sb = p1.tile([128, 6, E], bft)
                nc.scalar.dma_start(out=wh1_sb, in_=wh1p[:].rearrange("(a p) e -> p a e", p=128))
                wt1_sb = p1.tile([128, 6, E], bft)
                nc.scalar.dma_start(out=wt1_sb, in_=wt1[:].rearrange("(a p) e -> p a e", p=128))
                wh2_sb = p1.tile([128, 6, E], bft)
                nc.scalar.dma_start(out=wh2_sb, in_=wh2p[:].rearrange("(a p) e -> p a e", p=128))
                wt2_sb = p1.tile([128, 6, E], bft)
                nc.scalar.dma_start(out=wt2_sb, in_=wt2[:].rearrange("(a p) e -> p a e", p=128))
                bh_sb = p1.tile([128, 6], f32)
                nc.gpsimd.dma_start(out=bh_sb, in_=bh_p[:])
                bt_sb = p1.tile([128, 6], f32)
                nc.gpsimd.dma_start(out=bt_sb, in_=bt_t[:])

                # ---------- mention path: logsumexp entity embeddings ----------
                if ksub < 2:
                    raise _SubStop(seq_sb)
                ment_g = p1.tile([128, H], f32)
                nc.gpsimd.indirect_dma_start(
                    out=ment_g[:],
                    out_offset=None,
                    in_=seq_f32[:],
                    in_offset=bass.IndirectOffsetOnAxis(ap=mi_sb[:, 0:1], axis=0),
                )
                exp_g = p1.tile([128, H], bft)
                nc.scalar.activation(out=exp_g, in_=ment_g[:], func=AF.Exp)

                # eeT[h, ent] = ln(sum_m exp(ment)) via selector matmul (fuses
                # the mention-sum with the transpose to [h, ent] layout)
                eeT = p1.tile([128, 6, 30], bft)
                for ht in range(6):
                    tr = ps1.tile([128, 30], f32, tag="sm1", bufs=2)
                    nc.tensor.matmul(tr, lhsT=exp_g[:, 128 * ht : 128 * (ht + 1)], rhs=S2_sb[:], start=True, stop=True)
                    nc.scalar.activation(out=eeT[:, ht, :], in_=tr, func=AF.Ln)

                # hpartT [E'(perm), own-i 6] / tpartT [E, j 24]
                hpT = p1.tile([128, 6, IPC], bft)
                tpT = p1.tile([128, 6, NE], bft)
                for Et in range(6):
                    hp = ps1.tile([128, IPC], f32, tag="sm1", bufs=2)
                    for ht in range(6):
                        nc.tensor.matmul(
                            hp, lhsT=wh1_sb[:, ht, 128 * Et : 128 * (Et + 1)],
                            rhs=eeT[:, ht, 24:30], start=(ht == 0), stop=(ht == 5))
                    nc.scalar.copy(out=hpT[:, Et, :], in_=hp)
                    tp = ps1.tile([128, NE], f32, tag="sm1", bufs=2)
                    for ht in range(6):
                        nc.tensor.matmul(
                            tp, lhsT=wt1_sb[:, ht, 128 * Et : 128 * (Et + 1)],
                            rhs=eeT[:, ht, 0:24], start=(ht == 0), stop=(ht == 5))
                    nc.scalar.copy(out=tpT[:, Et, :], in_=tp)

                # ---------- attention path ----------
                if ksub < 3:
                    raise _SubStop(tpT)
                # e_att^T tiles [l, (i30, h12)] via selector matmuls (sums mentions)
                eaT = p1.tile([128, 4, 30, NH], bft)
                for h in range(NH):
                    for lt in range(4):
                        ep = ps1.tile([128, 30], f32, tag="sm1", bufs=2)
                        nc.tensor.matmul(
                            ep, lhsT=att_b[0:120, h, 128 * lt : 128 * (lt + 1)],
                            rhs=S_sb[:], start=True, stop=True)
                        nc.scalar.copy(out=eaT[:, lt, :, h], in_=ep)

                # ---------- pair attention products + normalization ----------
                if ksub < 4:
                    raise _SubStop(eaT)
                htn = p1.tile([128, 4, PL], bft)  # normalized ht_att^T per l-tile
                ht_raw = p1.tile([128, 4, PL], bft)
                sum_ps = ps1.tile([1, PL], f32, tag="lsum", bufs=1)
                with nc.allow_low_precision("bf16 pair-product reduce; normalization is scale-invariant"):
                    for lt in range(4):
                        prod = p1.tile([128, IPC, NE, NH], bft, tag="prod", bufs=2)
                        in0 = eaT[:, lt, 24:30, :].unsqueeze(2).broadcast_to([128, IPC, NE, NH])
                        in1 = eaT[:, lt, 0:24, :].unsqueeze(1).broadcast_to([128, IPC, NE, NH])
                        nc.vector.tensor_mul(out=prod, in0=in0, in1=in1)
                        nc.vector.tensor_reduce(
                            out=ht_raw[:, lt, :],
                            in_=prod[:].rearrange("p a b h -> p (a b) h"),
                            axis=mybir.AxisListType.X, op=ALU.add)
                        nc.vector.tensor_scalar_max(
                            out=ht_raw[:, lt, :], in0=ht_raw[:, lt, :], scalar1=0.0)
                        nc.tensor.matmul(sum_ps, lhsT=ones_sb[:], rhs=ht_raw[:, lt, :],
                                         start=(lt == 0), stop=(lt == 3))
                denom = p1.tile([1, PL], f32)
                nc.vector.tensor_scalar_add(out=denom, in0=sum_ps, scalar1=1e-10)
                recip = p1.tile([1, PL], f32)
                nc.vector.reciprocal(out=recip, in_=denom)
                rep_ps = ps1.tile([128, PL], f32, tag="acc", bufs=2)
                nc.tensor.matmul(rep_ps, lhsT=onesrow_sb[:], rhs=recip[:], start=True, stop=True)
                recip_rep = p1.tile([128, PL], f32)
                nc.vector.tensor_copy(out=recip_rep, in_=rep_ps)
                for lt in range(4):
                    nc.vector.tensor_mul(out=htn[:, lt, :], in0=ht_raw[:, lt, :], in1=recip_rep)

                # ---------- rs^T = seq^T @ ht_n ----------
                if ksub < 5:
                    raise _SubStop(htn)
                rsT = p1.tile([128, 6, PL], bft)
                for ht in range(6):
                    rp = ps1.tile([128, PL], f32, tag="acc", bufs=2)
                    for lt in range(4):
                        nc.tensor.matmul(rp, lhsT=seq_sb[:, lt, 128 * ht : 128 * (ht + 1)],
                                         rhs=htn[:, lt, :], start=(lt == 0), stop=(lt == 3))
                    nc.scalar.copy(out=rsT[:, ht, :], in_=rp)

                # ---------- extractors: ts first so its AllGather launches early ----------
                for Et in range(6):
                    tp_b = tpT[:, Et, :].unsqueeze(1).broadcast_to([128, IPC, NE])
                    ep2 = ps1.tile([128, PL], f32, tag="acc", bufs=2)
                    for ht in range(6):
                        nc.tensor.matmul(ep2, lhsT=wt2_sb[:, ht, 128 * Et : 128 * (Et + 1)],
                                         rhs=rsT[:, ht, :], start=(ht == 0), stop=(ht == 5))
                    nc.vector.tensor_add(out=ep2[:].rearrange("p (a b) -> p a b", a=IPC),
                                         in0=ep2[:].rearrange("p (a b) -> p a b", a=IPC), in1=tp_b)
                    nc.scalar.activation(out=ts_sb[:, Et, :], in_=ep2, func=AF.Tanh,
                                         bias=bt_sb[:, Et : Et + 1])
                for Et in range(6):
                    hp_b = hpT[:, Et, :].unsqueeze(2).broadcast_to([128, IPC, NE])
                    ep = ps1.tile([128, PL], f32, tag="acc", bufs=2)
                    for ht in range(6):
                        nc.tensor.matmul(ep, lhsT=wh2_sb[:, ht, 128 * Et : 128 * (Et + 1)],
                                         rhs=rsT[:, ht, :], start=(ht == 0), stop=(ht == 5))
                    nc.vector.tensor_add(out=ep[:].rearrange("p (a b) -> p a b", a=IPC),
                                         in0=ep[:].rearrange("p (a b) -> p a b", a=IPC), in1=hp_b)
                    nc.scalar.activation(out=hs_sb[:, Et, :], in_=ep, func=AF.Tanh,
                                         bias=bh_sb[:, Et : Et + 1])

            # ---------- collectives: redistribute hs (AllToAll), replicate ts ----------
              except _SubStop as e:
                t = e.tile_ap
                dmy = persist.tile([C, NP], f32)
                nc.vector.memset(dmy, 0.0)
                src = t[0:C]
                fs = 1
                for d in src.shape[1:]:
                    fs *= d
                fs = min(fs, NP)
                nc.vector.tensor_scalar_add(
                    out=dmy[:, 0:fs],
                    in0=src.rearrange(" ".join(["p"] + [chr(97+i) for i in range(len(src.shape)-1)]) + " -> p (" + " ".join(chr(97+i) for i in range(len(src.shape)-1)) + ")")[:, 0:fs],
                    scalar1=0.0)
                nc.sync.dma_start(out=out_lgT[:], in_=dmy[:])
                import os as _os
                _os.environ["KSTAGE"] = "0"
            stage = int(os.environ.get("KSTAGE", "4"))
            if stage >= 1:
                nc.sync.dma_start(
                    out=ts_cc_in[0 : E // 2, :].rearrange("(a p) c -> p a c", p=128),
                    in_=ts_sb[:, 0:3, :])
                nc.sync.dma_start(
                    out=ts_cc_in[E // 2 : E, :].rearrange("(a p) c -> p a c", p=128),
                    in_=ts_sb[:, 3:6, :])
                nc.sync.dma_start(out=hs_cc_in[:].rearrange("(a p) c -> p a c", p=128), in_=hs_sb[:])
            if stage == 1:
                dmy = persist.tile([C, NP], f32)
                nc.vector.memset(dmy, 0.0)
                nc.vector.tensor_scalar_add(out=dmy[:, 0:PL], in0=hs_sb[0:C, 0, :], scalar1=0.0)
                nc.sync.dma_start(out=out_lgT[:], in_=dmy[:])
            if stage >= 2:
                nc.gpsimd.collective_compute(
                    "AllGather", ALU.bypass, replica_groups=groups,
                    ins=[ts_cc_in[0 : E // 2, :].opt()], outs=[ts_cc_o1[:].opt()])
                nc.gpsimd.collective_compute(
                    "AllToAll", ALU.bypass, replica_groups=groups,
                    ins=[hs_cc_in[:].opt()], outs=[hs_cc_out[:].opt()])
                nc.gpsimd.collective_compute(
                    "AllGather", ALU.bypass, replica_groups=groups,
                    ins=[ts_cc_in[E // 2 : E, :].opt()], outs=[ts_cc_o2[:].opt()])
            if stage == 2:
                dmy = persist.tile([C, NP], f32)
                nc.vector.memset(dmy, 0.0)
                rb = persist.tile([C, PL], bft)
                nc.sync.dma_start(out=rb[:], in_=hs_cc_out[0:C, :])
                rb2 = persist.tile([C, PL], bft)
                nc.sync.dma_start(out=rb2[:], in_=ts_cc_o1[3, 0:C, :])
                nc.vector.tensor_add(out=dmy[:, 0:PL], in0=rb, in1=rb2)
                nc.sync.dma_start(out=out_lgT[:], in_=dmy[:])

            # ---------- phase 2: feature + classifier over pair chunks ----------
            with (
                tc.tile_pool(name="p2", bufs=2) as p2,
                tc.tile_pool(name="ps2", bufs=1, space="PSUM") as ps2,
            ):
                for ck in range(NCHUNK if stage >= 3 else 0):
                    # b2rep: per k, the ts k-block [64, CW] duplicated to 128 partitions
                    b2r = p2.tile([128, K, CW], bft, tag="b2r", bufs=2)
                    for k in range(K):
                        src = ts_cc_o1 if k < 6 else ts_cc_o2
                        kk = k if k < 6 else k - 6
                        for half in range(2):
                            nc.sync.dma_start(
                                out=b2r[64 * half : 64 * (half + 1), k, :].rearrange(
                                    "t (c d) -> t c d", c=2),
                                in_=bass.AP(
                                    tensor=src,
                                    offset=2 * ck * ((E // 2) * PL) + (kk * BS) * PL,
                                    ap=[[PL, BS], [(E // 2) * PL, 2], [1, PL]],
                                ),
                            )
                    fps = []
                    for h in range(6):
                        fpt = ps2.tile([128, CW], f32, tag=f"feat{h}", bufs=1, name=f"fps{h}")
                        fps.append(fpt)
                    hs2k = []
                    for k in range(K):
                        hkt = p2.tile([8, 2, PL], bft, tag="hs2", bufs=13, name=f"hs2_{k}")
                        nc.sync.dma_start(
                            out=hkt[:],
                            in_=bass.AP(
                                tensor=hs_cc_out,
                                offset=2 * ck * (96 * PL) + k * PL,
                                ap=[[12 * PL, 8], [96 * PL, 2], [1, PL]],
                            ),
                        )
                        hs2k.append(hkt)
                    for kt in range(NKT):
                        k, sp = kt // 4, kt % 4
                        b1ps = ps2.tile([128, CW], f32, tag="b1ps", bufs=2)
                        nc.tensor.matmul(
                            b1ps, lhsT=sel2_sb[:, sp, :],
                            rhs=hs2k[k][:].rearrange("a c d -> a (c d)"),
                            start=True, stop=True)
                        bl = p2.tile([128, CW], bft, tag="bl", bufs=3)
                        nc.vector.tensor_mul(out=bl, in0=b1ps, in1=b2r[:, k, :])
                        for h in range(6):
                            nc.tensor.matmul(
                                fps[h], lhsT=wp_sb[:, kt, 128 * h : 128 * (h + 1)],
                                rhs=bl, start=(kt == 0), stop=(kt == NKT - 1))
                    lgp = ps2.tile([C, CW], f32, tag="b1ps", bufs=2)
                    for h in range(6):
                        fT = p2.tile([128, CW], bft, tag="fT", bufs=2)
                        nc.scalar.copy(out=fT, in_=fps[h])
                        nc.tensor.matmul(lgp, lhsT=wc_sb[:, h, :], rhs=fT,
                                         start=(h == 0), stop=(h == 5))
                    nc.scalar.copy(out=lg_sb[:, ck * CW : (ck + 1) * CW], in_=lgp)
                    if stage >= 4:
                        nc.sync.dma_start(
                            out=lg_cc_in[ck, :, :],
                            in_=lg_sb[:, ck * CW : (ck + 1) * CW])
                        nc.gpsimd.collective_compute(
                            "AllReduce", ALU.add, replica_groups=groups,
                            ins=[lg_cc_in[ck, :, :].opt()],
                            outs=[lg_cc_out[ck, :, :].opt()])
                        lg_f = p2.tile([C, CW], f32, tag="lgf", bufs=2)
                        nc.sync.dma_start(out=lg_f[:], in_=lg_cc_out[ck, :, :])
                        nc.vector.tensor_scalar_add(out=lg_f, in0=lg_f, scalar1=bc_sb[:])
                        nc.vector.tensor_mul(
                            out=lg_f, in0=lg_f, in1=mask_sb[:, ck * CW : (ck + 1) * CW])
                        nc.sync.dma_start(
                            out=out_lgT[:, ck * CW : (ck + 1) * CW], in_=lg_f[:])

                # ---------- stage-3 debug output ----------
                if stage == 3:
                    nc.sync.dma_start(out=out_lgT[:], in_=lg_sb[:])

    if not nc.is_finalized():
        nc.finalize()
    return nc


_NC_CACHE = None


def kernel(**inputs):
    global _NC_CACHE
    from concourse.bass_utils import run_bass_kernel_spmd

    if _NC_CACHE is None:
        _NC_CACHE = build_bass()
    in_maps = _host_prep(inputs)
    res = run_bass_kernel_spmd(_NC_CACHE, in_maps, core_ids=list(range(NCORE)))
    kernel.last_results = res
    out = res.results[0]["out_lgT"]  # [97, 1152]
    return np.ascontiguousarray(out.T).astype(np.float32)



# revision 2
# speedup vs baseline: 1.1481x; 1.1481x over previous
"""Trainium2 Bass kernel v2 for nn_DocREModel_Triangle — collective-free.

Strategy (8 NeuronCores, SPMD, no collectives):
  Phase 1 (replicated): every core computes ALL 1152 (b,i,j) pairs —
  mention gather (host) + exp/selector-matmul/ln entity pooling, host-
  transposed attention gather + on-device mention-sum, DVE pair products
  + l-normalization, rs^T = seq^T @ htn, and the two tanh extractors.
  ts^T [768, 1152] is computed fully; hs^T only for the core's 96
  (k, s)-rows (s in [8c, 8c+8)), via a host column-permutation of Wh.
  Phase 2 (contraction-split): core c holds Wp rows (s in its slice,
  k, t) resident in SBUF (bf16, 9.4 MB) and accumulates
  feature^T partial = sum over its 6144 (s,k,t) rows of
  hs[k,s]*ts[k,t] * Wp[(k,s,t), :] for all 1152 pairs, then the
  classifier -> partial logits [97, 1152] DMA'd out per chunk.
  The 8 partial logits are summed on the HOST (gather/unshard step);
  bias + self-mask also applied on host. No AllReduce, no barrier.

  Phase-2 tiling: tile kt2 = s*6 + a covers contraction rows
  (k in {2a, 2a+1}, t in 0..63) for one s: the ts operand is the
  natural E-layout SBUF slice ts_sb[:, a, :] (no replication needed);
  the hs row-pair is broadcast 2x64 partitions by a stride-0 DMA from
  a local DRAM bounce (hs_dr). PE streams 6 feature matmuls per tile
  back-to-back (N=384), LDWEIGHTS hidden, no PE->DVE->PE ping-pong.
"""

import numpy as np
import ml_dtypes

bf16 = ml_dtypes.bfloat16

B, L, H, NH = 2, 512, 768, 12
NE, NM = 24, 4
E, BS, C = 768, 64, 97
K = E // BS                      # 12 blocks
NCORE = 8
SL = 64 // NCORE                 # 8 s-values per core
NP = B * NE * NE                 # 1152 pairs
PPB = NE * NE                    # 576 pairs per batch
CW = 288                         # phase-1 pair chunk (12 i x 24 j)
NCH1 = NP // CW                  # 4
CW2 = 384                        # phase-2 pair chunk (psum-bank sized)
NCH2 = NP // CW2                 # 3
KST = K * SL * BS                # 6144 contraction rows per core
NT2 = KST // 128                 # 48 tiles, kt2 = s*6 + a (a = k//2)


def _host_prep(inputs):
    """Build the 8 per-core input maps from the full inputs."""
    seq = np.ascontiguousarray(inputs["sequence_output"], dtype=np.float32)
    att = np.ascontiguousarray(inputs["attention"], dtype=np.float32)
    Wh = np.asarray(inputs["Wh"], dtype=np.float32)
    Wt = np.asarray(inputs["Wt"], dtype=np.float32)
    Wp = np.asarray(inputs["Wp"], dtype=np.float32)
    Wc = np.asarray(inputs["Wc"], dtype=np.float32)
    bh = np.asarray(inputs["bh"], dtype=np.float32)
    bt = np.asarray(inputs["bt"], dtype=np.float32)
    mpos = np.asarray(inputs["mention_pos"]).astype(np.int64)

    seq_bf = np.ascontiguousarray(seq.astype(bf16))
    wt1 = np.ascontiguousarray(Wt[:H].astype(bf16))
    wt2 = np.ascontiguousarray(Wt[H:].astype(bf16))
    bt_t = np.ascontiguousarray(bt.reshape(6, 128).T.astype(np.float32))
    wc_bf = np.ascontiguousarray(Wc.astype(bf16))
    wp4 = Wp.reshape(K, 64, BS, H)

    # mention rows (m, e): row = m*24 + e  -> [96, 2, 768] f32
    ment = np.ascontiguousarray(
        np.stack([seq[b][mpos[b].T.reshape(-1)] for b in range(B)], 1)
        .astype(np.float32))
    # attention gather, transposed: [b, l, (e h m)] bf16
    attT = np.zeros((B, L, NE * NH * NM), bf16)
    for b in range(B):
        G = att[b][:, mpos[b].reshape(-1), :]                # [12, 96, 512]
        X = G.reshape(NH, NE, NM, L).transpose(3, 1, 0, 2)   # [512, 24, 12, 4]
        attT[b] = X.reshape(L, -1).astype(bf16)
    attT = np.ascontiguousarray(attT)

    in_maps = []
    for c in range(NCORE):
        PERM_c = np.array([k * 64 + SL * c + sl for k in range(K) for sl in range(SL)])
        wh1p = np.ascontiguousarray(Wh[:H][:, PERM_c].astype(bf16))   # [768, 96]
        wh2p = np.ascontiguousarray(Wh[H:][:, PERM_c].astype(bf16))
        bh_c = np.ascontiguousarray(bh[PERM_c].reshape(96, 1).astype(np.float32))
        wp_c = np.ascontiguousarray(
            wp4[:, SL * c: SL * (c + 1)].transpose(1, 0, 2, 3)
            .reshape(KST, H).astype(bf16))
        in_maps.append({
            "seq_bf": seq_bf,
            "att_T": attT,
            "ment": ment,
            "wh1p": wh1p,
            "wh2p": wh2p,
            "wt1": wt1,
            "wt2": wt2,
            "bh_c": bh_c,
            "bt_t": bt_t,
            "wp_sl": wp_c,
            "wc_bf": wc_bf,
        })
    return in_maps


def _build_consts():
    # selector [96, 24]: sums the 4 mention rows per entity during the
    # transpose-matmul of exp(ment) (rows are (m, e), m-major)
    S2 = np.zeros((96, NE), dtype=bf16)
    for m in range(NM):
        for e in range(NE):
            S2[m * NE + e, e] = 1.0
    ones_bf = np.ones((128, 1), dtype=bf16)
    ones_row = np.ones((1, 128), dtype=np.float32)
    return S2, ones_bf, ones_row


def build_bass():
    import concourse.bass as bass
    import concourse.mybir as mybir
    import concourse.tile as tile
    from concourse.bacc import Bacc

    f32 = mybir.dt.float32
    bft = mybir.dt.bfloat16
    AF = mybir.ActivationFunctionType
    ALU = mybir.AluOpType
    X = mybir.AxisListType.X

    nc = Bacc("TRN2", num_devices=NCORE)

    # ---- I/O ----
    seq_bf = nc.dram_tensor("seq_bf", [B, L, H], bft, kind="ExternalInput")
    att_T = nc.dram_tensor("att_T", [B, L, NE * NH * NM], bft, kind="ExternalInput")
    ment = nc.dram_tensor("ment", [96, B, H], f32, kind="ExternalInput")
    wh1p = nc.dram_tensor("wh1p", [H, 96], bft, kind="ExternalInput")
    wh2p = nc.dram_tensor("wh2p", [H, 96], bft, kind="ExternalInput")
    wt1 = nc.dram_tensor("wt1", [H, E], bft, kind="ExternalInput")
    wt2 = nc.dram_tensor("wt2", [H, E], bft, kind="ExternalInput")
    bh_c = nc.dram_tensor("bh_c", [96, 1], f32, kind="ExternalInput")
    bt_t = nc.dram_tensor("bt_t", [128, 6], f32, kind="ExternalInput")
    wp_sl = nc.dram_tensor("wp_sl", [KST, H], bft, kind="ExternalInput")
    wc_bf = nc.dram_tensor("wc_bf", [H, C], bft, kind="ExternalInput")
    out_lgT = nc.dram_tensor("out_lgT", [C, NP], f32, kind="ExternalOutput")

    hs_dr = nc.dram_tensor("hs_dr", [96, NP], bft)   # local bounce for hs^T

    S2_np, ones_np, onesrow_np = _build_consts()
    S2_dr = nc.inline_tensor(S2_np, "s2_const")
    ones_dr = nc.inline_tensor(ones_np, "ones_const")
    onesrow_dr = nc.inline_tensor(onesrow_np, "onesrow_const")

    with tile.TileContext(nc) as tc:
        with tc.tile_pool(name="gpool", bufs=1) as gpool:
            # ---------- whole-kernel-lifetime tiles ----------
            wp_sb = gpool.tile([128, NT2, H], bft)
            wc_sb = gpool.tile([128, 6, C], bft)
            ts_sb = gpool.tile([128, 6, NP], bft)
            bt_sb = gpool.tile([128, 6], f32)
            bh_sb = gpool.tile([96, 1], f32)
            ones_sb = gpool.tile([128, 1], bft)
            onesrow_sb = gpool.tile([1, 128], f32)
            nc.gpsimd.dma_start(out=bt_sb, in_=bt_t[:])
            nc.gpsimd.dma_start(out=bh_sb, in_=bh_c[:])
            nc.gpsimd.dma_start(out=ones_sb, in_=ones_dr[:])
            nc.gpsimd.dma_start(out=onesrow_sb, in_=onesrow_dr[:])

            with (
                tc.tile_pool(name="p1", bufs=1) as p1,
                tc.tile_pool(name="ps1", bufs=1, space="PSUM") as ps1,
            ):
                # ---------- inputs, critical path first ----------
                ment_sb = p1.tile([96, B, H], f32)
                nc.gpsimd.dma_start(out=ment_sb, in_=ment[:])
                S2_sb = p1.tile([96, NE], bft)
                nc.gpsimd.dma_start(out=S2_sb, in_=S2_dr[:])
                att_sb = p1.tile([128, 4 * B, NE * NH * NM], bft)
                seq_sb = p1.tile([128, 4 * B, H], bft)
                for b in range(B):
                    nc.sync.dma_start(
                        out=att_sb[:, 4 * b: 4 * b + 4, :],
                        in_=att_T[b].rearrange("(a p) x -> p a x", p=128))
                    nc.sync.dma_start(
                        out=seq_sb[:, 4 * b: 4 * b + 4, :],
                        in_=seq_bf[b].rearrange("(a p) h -> p a h", p=128))
                wh1_sb = p1.tile([128, 6, 96], bft)
                nc.scalar.dma_start(out=wh1_sb, in_=wh1p[:].rearrange("(a p) e -> p a e", p=128))
                wt1_sb = p1.tile([128, 6, E], bft)
                nc.scalar.dma_start(out=wt1_sb, in_=wt1[:].rearrange("(a p) e -> p a e", p=128))
                wh2_sb = p1.tile([128, 6, 96], bft)
                nc.scalar.dma_start(out=wh2_sb, in_=wh2p[:].rearrange("(a p) e -> p a e", p=128))
                wt2_sb = p1.tile([128, 6, E], bft)
                nc.scalar.dma_start(out=wt2_sb, in_=wt2[:].rearrange("(a p) e -> p a e", p=128))
                # background loads for phase 2
                nc.sync.dma_start(out=wp_sb, in_=wp_sl[:].rearrange("(a p) h -> p a h", p=128))
                nc.scalar.dma_start(out=wc_sb, in_=wc_bf[:].rearrange("(a p) c -> p a c", p=128))

                # ---------- entity pooling: eeT = ln(sum_m exp(ment)) ----------
                exp_sb = p1.tile([96, B, H], bft)
                for b in range(B):
                    nc.scalar.activation(out=exp_sb[:, b, :], in_=ment_sb[:, b, :], func=AF.Exp)
                eeT = p1.tile([128, 6, B * NE], bft)    # cols (b, ent)
                for ht in range(6):
                    for b in range(B):
                        tr = ps1.tile([128, 2 * NE], f32, tag="mm48", bufs=2)
                        nc.tensor.matmul(tr[:, 0:NE], lhsT=exp_sb[:, b, 128 * ht: 128 * (ht + 1)],
                                         rhs=S2_sb[:], start=True, stop=True)
                        nc.scalar.activation(out=eeT[:, ht, NE * b: NE * (b + 1)],
                                             in_=tr[:, 0:NE], func=AF.Ln)

                # ---------- hpT [96, 48] / tpT [128, 6, 48] ----------
                hp_ps = ps1.tile([128, 2 * NE], f32, tag="mm48", bufs=2)
                for ht in range(6):
                    nc.tensor.matmul(hp_ps[0:96, :], lhsT=wh1_sb[:, ht, :], rhs=eeT[:, ht, :],
                                     start=(ht == 0), stop=(ht == 5))
                hpT = p1.tile([96, B * NE], bft)
                nc.scalar.copy(out=hpT, in_=hp_ps[0:96, :])
                tpT = p1.tile([128, 6, B * NE], bft)
                for Et in range(6):
                    tp_ps = ps1.tile([128, 2 * NE], f32, tag="mm48", bufs=2)
                    for ht in range(6):
                        nc.tensor.matmul(tp_ps, lhsT=wt1_sb[:, ht, 128 * Et: 128 * (Et + 1)],
                                         rhs=eeT[:, ht, :], start=(ht == 0), stop=(ht == 5))
                    nc.scalar.copy(out=tpT[:, Et, :], in_=tp_ps)

                # ---------- attention: mention-sum (the /4 cancels) ----------
                eaT = p1.tile([128, 4 * B, NE * NH], bft)
                with nc.allow_low_precision("4-term mention-sum in bf16; feeds scale-invariant normalization"):
                    for blt in range(4 * B):
                        nc.vector.tensor_reduce(
                            out=eaT[:, blt, :],
                            in_=att_sb[:, blt, :].rearrange("p (x m) -> p x m", m=NM),
                            axis=X, op=ALU.add)

                # ---------- pair products + l-normalization ----------
                ht_raw = p1.tile([128, 4, NP], bft)
                with nc.allow_low_precision("bf16 pair-product reduce; normalization is scale-invariant"):
                    for b in range(B):
                        for lt in range(4):
                            blt = 4 * b + lt
                            for ih in range(2):
                                prod = p1.tile([128, 12, NE, NH], bft, tag="prod", bufs=2)
                                in0 = (eaT[:, blt, :].rearrange("p (e h) -> p e h", h=NH)
                                       [:, 12 * ih: 12 * (ih + 1), :]
                                       .unsqueeze(2).broadcast_to([128, 12, NE, NH]))
                                in1 = (eaT[:, blt, :].rearrange("p (e h) -> p e h", h=NH)
                                       .unsqueeze(1).broadcast_to([128, 12, NE, NH]))
                                nc.vector.tensor_mul(out=prod, in0=in0, in1=in1)
                                nc.vector.tensor_reduce(
                                    out=ht_raw[:, lt, b * PPB + CW * ih: b * PPB + CW * (ih + 1)],
                                    in_=prod[:].rearrange("p a b h -> p (a b) h"),
                                    axis=X, op=ALU.add)
                    nc.vector.tensor_scalar_max(out=ht_raw[:], in0=ht_raw[:], scalar1=0.0)

                htn = p1.tile([128, 4, NP], bft)
                for ck in range(NCH1):
                    sl1 = slice(CW * ck, CW * (ck + 1))
                    sum_ps = ps1.tile([128, CW], f32, tag="mm288", bufs=3)
                    for lt in range(4):
                        nc.tensor.matmul(sum_ps[0:1, :], lhsT=ones_sb[:], rhs=ht_raw[:, lt, sl1],
                                         start=(lt == 0), stop=(lt == 3))
                    recip = p1.tile([1, CW], f32, tag="recip", bufs=2)
                    nc.vector.tensor_scalar_add(out=recip, in0=sum_ps[0:1, :], scalar1=1e-10)
                    nc.vector.reciprocal(out=recip, in_=recip)
                    rep_ps = ps1.tile([128, CW], f32, tag="mm288", bufs=3)
                    nc.tensor.matmul(rep_ps, lhsT=onesrow_sb[:], rhs=recip[:], start=True, stop=True)
                    rrep = p1.tile([128, CW], f32, tag="rrep", bufs=2)
                    nc.vector.tensor_copy(out=rrep, in_=rep_ps)
                    for lt in range(4):
                        nc.vector.tensor_mul(out=htn[:, lt, sl1], in0=ht_raw[:, lt, sl1], in1=rrep)

                # ---------- rsT = seq^T @ htn ----------
                rsT = p1.tile([128, 6, NP], bft)
                for ht in range(6):
                    for ck in range(NCH1):
                        b = ck // 2
                        rp = ps1.tile([128, CW], f32, tag="mm288", bufs=3)
                        for lt in range(4):
                            nc.tensor.matmul(rp, lhsT=seq_sb[:, 4 * b + lt, 128 * ht: 128 * (ht + 1)],
                                             rhs=htn[:, lt, CW * ck: CW * (ck + 1)],
                                             start=(lt == 0), stop=(lt == 3))
                        nc.scalar.copy(out=rsT[:, ht, CW * ck: CW * (ck + 1)], in_=rp)

                # ---------- hs extractor first (phase 2 needs hs_dr) ----------
                for ck in range(NCH1):
                    b, ih = ck // 2, ck % 2
                    hp3 = ps1.tile([128, CW], f32, tag="mm288", bufs=3)
                    for ht in range(6):
                        nc.tensor.matmul(hp3[0:96, :], lhsT=wh2_sb[:, ht, :],
                                         rhs=rsT[:, ht, CW * ck: CW * (ck + 1)],
                                         start=(ht == 0), stop=(ht == 5))
                    hpb = (hpT[:, NE * b + 12 * ih: NE * b + 12 * (ih + 1)]
                           .unsqueeze(2).broadcast_to([96, 12, NE]))
                    nc.vector.tensor_add(out=hp3[0:96, :].rearrange("p (i j) -> p i j", i=12),
                                         in0=hp3[0:96, :].rearrange("p (i j) -> p i j", i=12),
                                         in1=hpb)
                    hs_t = p1.tile([96, CW], bft, tag="hst", bufs=2)
                    nc.scalar.activation(out=hs_t, in_=hp3[0:96, :], func=AF.Tanh,
                                         bias=bh_sb[:, 0:1])
                    nc.sync.dma_start(out=hs_dr[:, CW * ck: CW * (ck + 1)], in_=hs_t[:])

                # ---------- ts extractor ----------
                for Et in range(6):
                    for ck in range(NCH1):
                        b, ih = ck // 2, ck % 2
                        ep = ps1.tile([128, CW], f32, tag="mm288", bufs=3)
                        for ht in range(6):
                            nc.tensor.matmul(ep, lhsT=wt2_sb[:, ht, 128 * Et: 128 * (Et + 1)],
                                             rhs=rsT[:, ht, CW * ck: CW * (ck + 1)],
                                             start=(ht == 0), stop=(ht == 5))
                        tpb = (tpT[:, Et, NE * b: NE * (b + 1)]
                               .unsqueeze(1).broadcast_to([128, 12, NE]))
                        nc.vector.tensor_add(out=ep[:].rearrange("p (i j) -> p i j", i=12),
                                             in0=ep[:].rearrange("p (i j) -> p i j", i=12),
                                             in1=tpb)
                        nc.scalar.activation(out=ts_sb[:, Et, CW * ck: CW * (ck + 1)],
                                             in_=ep, func=AF.Tanh, bias=bt_sb[:, Et: Et + 1])

            # ---------- phase 2: bilinear + projection + classifier ----------
            with (
                tc.tile_pool(name="p2", bufs=1) as p2,
                tc.tile_pool(name="ps2", bufs=1, space="PSUM") as ps2,
            ):
                for ck in range(NCH2):
                    sl2 = slice(CW2 * ck, CW2 * (ck + 1))
                    fps = []
                    for h in range(6):
                        fps.append(ps2.tile([128, CW2], f32, tag=f"feat{h}", bufs=1,
                                            name=f"fps{h}"))
                    for kt2 in range(NT2):
                        s_l, a = kt2 // 6, kt2 % 6
                        r0 = (2 * a) * SL + s_l
                        r1 = (2 * a + 1) * SL + s_l
                        b1r = p2.tile([128, CW2], bft, tag="b1r", bufs=6)
                        nc.sync.dma_start(
                            out=b1r[0:64, :],
                            in_=bass.AP(tensor=hs_dr, offset=r0 * NP + CW2 * ck,
                                        ap=[[0, 64], [1, CW2]]))
                        nc.sync.dma_start(
                            out=b1r[64:128, :],
                            in_=bass.AP(tensor=hs_dr, offset=r1 * NP + CW2 * ck,
                                        ap=[[0, 64], [1, CW2]]))
                        bl = p2.tile([128, CW2], bft, tag="bl", bufs=6)
                        nc.vector.tensor_mul(out=bl, in0=b1r, in1=ts_sb[:, a, sl2])
                        for h in range(6):
                            nc.tensor.matmul(fps[h], lhsT=wp_sb[:, kt2, 128 * h: 128 * (h + 1)],
                                             rhs=bl, start=(kt2 == 0), stop=(kt2 == NT2 - 1))
                    lgp = ps2.tile([C, CW2], f32, tag="lg", bufs=2)
                    for h in range(6):
                        fT = p2.tile([128, CW2], bft, tag="fT", bufs=3)
                        if h % 2 == 0:
                            nc.scalar.copy(out=fT, in_=fps[h])
                        else:
                            nc.vector.tensor_copy(out=fT, in_=fps[h])
                        nc.tensor.matmul(lgp, lhsT=wc_sb[:, h, :], rhs=fT,
                                         start=(h == 0), stop=(h == 5))
                    lgo = p2.tile([C, CW2], f32, tag="lgo", bufs=2)
                    nc.vector.tensor_copy(out=lgo, in_=lgp)
                    nc.sync.dma_start(out=out_lgT[:, sl2], in_=lgo[:])

    if not nc.is_finalized():
        nc.finalize()
    return nc


_NC_CACHE = None

def _host_finish(partials, bc):
    lgT = np.sum([np.asarray(p, dtype=np.float32) for p in partials], axis=0)
    logits = np.ascontiguousarray(lgT.T) + bc[None, :]
    mask = np.ones((NP, 1), np.float32)
    for b in range(B):
        for i in range(NE):
            mask[b * PPB + i * NE + i, 0] = 0.0
    return (logits * mask).astype(np.float32)


def kernel(**inputs):
    global _NC_CACHE
    from concourse.bass_utils import run_bass_kernel_spmd

    if _NC_CACHE is None:
        _NC_CACHE = build_bass()
    in_maps = _host_prep(inputs)
    res = run_bass_kernel_spmd(_NC_CACHE, in_maps, core_ids=list(range(NCORE)))
    kernel.last_results = res
    bc = np.asarray(inputs["bc"], dtype=np.float32)
    return _host_finish([res.results[c]["out_lgT"] for c in range(NCORE)], bc)


# revision 3
# speedup vs baseline: 1.3453x; 1.1717x over previous
"""Trainium2 Bass kernel v3 for nn_DocREModel_Triangle — collective-free.

v3 over v2 (trace-driven):
  - mention-sum via 3 contiguous DVE adds (host att layout [l, m, e, h])
    instead of a short-window tensor_reduce (was 10.3us at 0.5x rate).
  - extractor hpart/tpart broadcast-adds folded into the PE accumulation
    as 0/1 indicator matmuls (tpJ/hpJ stored [ent, E]) — removes 28 DVE
    broadcast-pattern adds (~13us).
  - normalization: (x + 1e-10) -> 1/x fused into one scalar-engine
    Reciprocal activation on the 128-replicated tile (DVE reciprocal on
    a 1-partition AP was 1.9us each).
  - phase-1 emitted chunk-major with per-chunk ht_raw/htn/rsT tiles so
    the PE pipeline (norm/rsT/extractors of chunk k) runs under the DVE
    products of chunk k+1 (was: single tiles -> full barrier).
  - phase 2: b1 hs-broadcast loads bulk-prefetched per chunk into b1c
    [128, 48, 384] (96 DMAs up-front, 3 issuing queues) instead of
    2 just-in-time DMAs per tile (64-descriptor stride-0 loads starved
    the PE to 64% occupancy).
  - A/B experiments in-run: product h-reduce via tensor_reduce (chunks
    0,1) vs pool_avg (chunks 2,3); product muls on gpsimd for chunk 3;
    phase-2 bl muls kt2%3==2 on gpsimd.
"""

import numpy as np
import ml_dtypes

bf16 = ml_dtypes.bfloat16

B, L, H, NH = 2, 512, 768, 12
NE, NM = 24, 4
E, BS, C = 768, 64, 97
K = E // BS
NCORE = 8
SL = 64 // NCORE                 # 8 s-values per core
NP = B * NE * NE                 # 1152 pairs
PPB = NE * NE                    # 576 pairs per batch
CW = 288                         # phase-1 pair chunk (12 i x 24 j)
NCH1 = NP // CW                  # 4
CW2 = 384                        # phase-2 pair chunk (psum-bank sized)
NCH2 = NP // CW2                 # 3
KST = K * SL * BS                # 6144 contraction rows per core
NT2 = KST // 128                 # 48 tiles, kt2 = s*6 + a (a = k//2)


def _host_prep(inputs):
    seq = np.ascontiguousarray(inputs["sequence_output"], dtype=np.float32)
    att = np.ascontiguousarray(inputs["attention"], dtype=np.float32)
    Wh = np.asarray(inputs["Wh"], dtype=np.float32)
    Wt = np.asarray(inputs["Wt"], dtype=np.float32)
    Wp = np.asarray(inputs["Wp"], dtype=np.float32)
    Wc = np.asarray(inputs["Wc"], dtype=np.float32)
    bh = np.asarray(inputs["bh"], dtype=np.float32)
    bt = np.asarray(inputs["bt"], dtype=np.float32)
    mpos = np.asarray(inputs["mention_pos"]).astype(np.int64)

    seq_bf = np.ascontiguousarray(seq.astype(bf16))
    wt1 = np.ascontiguousarray(Wt[:H].astype(bf16))
    wt2 = np.ascontiguousarray(Wt[H:].astype(bf16))
    bt_t = np.ascontiguousarray(bt.reshape(6, 128).T.astype(np.float32))
    wc_bf = np.ascontiguousarray(Wc.astype(bf16))
    wp4 = Wp.reshape(K, 64, BS, H)

    # mention rows (m, e): row = m*24 + e  -> [96, 2, 768] bf16
    ment = np.ascontiguousarray(
        np.stack([seq[b][mpos[b].T.reshape(-1)] for b in range(B)], 1)
        .astype(bf16))
    # attention gather, transposed, m-major: [b, l, (m e h)] bf16
    attT = np.zeros((B, L, NM * NE * NH), bf16)
    for b in range(B):
        G = att[b][:, mpos[b].reshape(-1), :]                 # [12, 96, 512]
        X = G.reshape(NH, NE, NM, L).transpose(3, 2, 1, 0)    # [512, 4m, 24e, 12h]
        attT[b] = X.reshape(L, -1).astype(bf16)
    attT = np.ascontiguousarray(attT)

    in_maps = []
    for c in range(NCORE):
        PERM_c = np.array([k * 64 + SL * c + sl for k in range(K) for sl in range(SL)])
        wh1p = np.ascontiguousarray(Wh[:H][:, PERM_c].astype(bf16))   # [768, 96]
        wh2p = np.ascontiguousarray(Wh[H:][:, PERM_c].astype(bf16))
        bh_c = np.ascontiguousarray(bh[PERM_c].reshape(96, 1).astype(np.float32))
        wp_c = np.ascontiguousarray(
            wp4[:, SL * c: SL * (c + 1)].transpose(1, 0, 2, 3)
            .reshape(KST, H).astype(bf16))
        in_maps.append({
            "seq_bf": seq_bf,
            "att_T": attT,
            "ment": ment,
            "wh1p": wh1p,
            "wh2p": wh2p,
            "wt1": wt1,
            "wt2": wt2,
            "bh_c": bh_c,
            "bt_t": bt_t,
            "wp_sl": wp_c,
            "wc_bf": wc_bf,
        })
    return in_maps


def _build_consts():
    # selector [96, 24]: sums the 4 mention rows per entity (rows (m, e))
    S2 = np.zeros((96, NE), dtype=bf16)
    for m in range(NM):
        for e in range(NE):
            S2[m * NE + e, e] = 1.0
    ones_bf = np.ones((128, 1), dtype=bf16)
    ones_row = np.ones((1, 128), dtype=np.float32)
    # j-indicator [24, 288]: J[j', il*24 + j] = (j == j')
    Jind = np.zeros((NE, CW), dtype=bf16)
    for il in range(12):
        for j in range(NE):
            Jind[j, il * NE + j] = 1.0
    # i-indicator [24, 2*288]: I[i', ih*288 + il*24 + j] = (i' == ih*12 + il)
    Iind = np.zeros((NE, 2 * CW), dtype=bf16)
    for ih in range(2):
        for il in range(12):
            for j in range(NE):
                Iind[ih * 12 + il, ih * CW + il * NE + j] = 1.0
    return S2, ones_bf, ones_row, Jind, Iind


def build_bass():
    import concourse.bass as bass
    import concourse.mybir as mybir
    import concourse.tile as tile
    from concourse.bacc import Bacc

    f32 = mybir.dt.float32
    bft = mybir.dt.bfloat16
    AF = mybir.ActivationFunctionType
    ALU = mybir.AluOpType
    X = mybir.AxisListType.X

    nc = Bacc("TRN2", num_devices=NCORE)

    seq_bf = nc.dram_tensor("seq_bf", [B, L, H], bft, kind="ExternalInput")
    att_T = nc.dram_tensor("att_T", [B, L, NM * NE * NH], bft, kind="ExternalInput")
    ment = nc.dram_tensor("ment", [96, B, H], bft, kind="ExternalInput")
    wh1p = nc.dram_tensor("wh1p", [H, 96], bft, kind="ExternalInput")
    wh2p = nc.dram_tensor("wh2p", [H, 96], bft, kind="ExternalInput")
    wt1 = nc.dram_tensor("wt1", [H, E], bft, kind="ExternalInput")
    wt2 = nc.dram_tensor("wt2", [H, E], bft, kind="ExternalInput")
    bh_c = nc.dram_tensor("bh_c", [96, 1], f32, kind="ExternalInput")
    bt_t = nc.dram_tensor("bt_t", [128, 6], f32, kind="ExternalInput")
    wp_sl = nc.dram_tensor("wp_sl", [KST, H], bft, kind="ExternalInput")
    wc_bf = nc.dram_tensor("wc_bf", [H, C], bft, kind="ExternalInput")
    out_lgT = nc.dram_tensor("out_lgT", [C, NP], f32, kind="ExternalOutput")

    hs_dr = nc.dram_tensor("hs_dr", [96, NP], bft)   # local bounce for hs^T

    S2_np, ones_np, onesrow_np, J_np, I_np = _build_consts()
    S2_dr = nc.inline_tensor(S2_np, "s2_const")
    ones_dr = nc.inline_tensor(ones_np, "ones_const")
    onesrow_dr = nc.inline_tensor(onesrow_np, "onesrow_const")
    J_dr = nc.inline_tensor(J_np, "j_const")
    I_dr = nc.inline_tensor(I_np, "i_const")

    with tile.TileContext(nc) as tc:
        with tc.tile_pool(name="gpool", bufs=1) as gpool:
            wp_sb = gpool.tile([128, NT2, H], bft)
            wc_sb = gpool.tile([128, 6, C], bft)
            ts_sb = gpool.tile([128, 6, NP], bft)
            tpJ = [gpool.tile([NE, E], bft, name=f"tpJ{b}") for b in range(B)]
            hpJ = [gpool.tile([NE, 96], bft, name=f"hpJ{b}") for b in range(B)]
            bt_sb = gpool.tile([128, 6], f32)
            bh_sb = gpool.tile([96, 1], f32)
            ones_sb = gpool.tile([128, 1], bft)
            onesrow_sb = gpool.tile([1, 128], f32)
            J_sb = gpool.tile([NE, CW], bft)
            I_sb = gpool.tile([NE, 2 * CW], bft)
            nc.gpsimd.dma_start(out=bt_sb, in_=bt_t[:])
            nc.gpsimd.dma_start(out=bh_sb, in_=bh_c[:])
            nc.gpsimd.dma_start(out=ones_sb, in_=ones_dr[:])
            nc.gpsimd.dma_start(out=onesrow_sb, in_=onesrow_dr[:])
            nc.gpsimd.dma_start(out=J_sb, in_=J_dr[:])
            nc.gpsimd.dma_start(out=I_sb, in_=I_dr[:])

            with (
                tc.tile_pool(name="p1", bufs=1) as p1,
                tc.tile_pool(name="ps1", bufs=1, space="PSUM") as ps1,
            ):
                # ---------- inputs, critical path first ----------
                ment_sb = p1.tile([96, B, H], bft)
                nc.gpsimd.dma_start(out=ment_sb, in_=ment[:])
                S2_sb = p1.tile([96, NE], bft)
                nc.gpsimd.dma_start(out=S2_sb, in_=S2_dr[:])
                att_sb = p1.tile([128, 4 * B, NM, NE * NH], bft)
                seq_sb = p1.tile([128, 4 * B, H], bft)
                for b in range(B):
                    nc.sync.dma_start(
                        out=att_sb[:, 4 * b: 4 * b + 4, :, :],
                        in_=att_T[b].rearrange("(a p) (m x) -> p a m x", p=128, m=NM))
                    nc.sync.dma_start(
                        out=seq_sb[:, 4 * b: 4 * b + 4, :],
                        in_=seq_bf[b].rearrange("(a p) h -> p a h", p=128))
                wh1_sb = p1.tile([128, 6, 96], bft)
                nc.scalar.dma_start(out=wh1_sb, in_=wh1p[:].rearrange("(a p) e -> p a e", p=128))
                wt1_sb = p1.tile([128, 6, E], bft)
                nc.scalar.dma_start(out=wt1_sb, in_=wt1[:].rearrange("(a p) e -> p a e", p=128))
                wh2_sb = p1.tile([128, 6, 96], bft)
                nc.scalar.dma_start(out=wh2_sb, in_=wh2p[:].rearrange("(a p) e -> p a e", p=128))
                wt2_sb = p1.tile([128, 6, E], bft)
                nc.scalar.dma_start(out=wt2_sb, in_=wt2[:].rearrange("(a p) e -> p a e", p=128))
                # background loads for phase 2
                nc.sync.dma_start(out=wp_sb, in_=wp_sl[:].rearrange("(a p) h -> p a h", p=128))
                nc.scalar.dma_start(out=wc_sb, in_=wc_bf[:].rearrange("(a p) c -> p a c", p=128))

                # ---------- entity pooling: eeT = ln(sum_m exp(ment)) ----------
                exp_sb = p1.tile([96, B, H], bft)
                for b in range(B):
                    nc.scalar.activation(out=exp_sb[:, b, :], in_=ment_sb[:, b, :], func=AF.Exp)
                eeT = p1.tile([128, 6, B * NE], bft)
                for ht in range(6):
                    for b in range(B):
                        tr = ps1.tile([128, B * NE], f32, tag="mm48", bufs=2)
                        nc.tensor.matmul(tr[:, 0:NE], lhsT=exp_sb[:, b, 128 * ht: 128 * (ht + 1)],
                                         rhs=S2_sb[:], start=True, stop=True)
                        nc.scalar.activation(out=eeT[:, ht, NE * b: NE * (b + 1)],
                                             in_=tr[:, 0:NE], func=AF.Ln)

                # ---------- tpJ / hpJ [24, E] per batch (ent-major) ----------
                for b in range(B):
                    for half in range(2):
                        tp_ps = ps1.tile([NE, CW2], f32, tag="tpj", bufs=2)
                        for ht in range(6):
                            nc.tensor.matmul(tp_ps, lhsT=eeT[:, ht, NE * b: NE * (b + 1)],
                                             rhs=wt1_sb[:, ht, CW2 * half: CW2 * (half + 1)],
                                             start=(ht == 0), stop=(ht == 5))
                        nc.scalar.copy(out=tpJ[b][:, CW2 * half: CW2 * (half + 1)], in_=tp_ps)
                    hp_ps = ps1.tile([NE, CW2], f32, tag="tpj", bufs=2)
                    for ht in range(6):
                        nc.tensor.matmul(hp_ps[:, 0:96], lhsT=eeT[:, ht, NE * b: NE * (b + 1)],
                                         rhs=wh1_sb[:, ht, :], start=(ht == 0), stop=(ht == 5))
                    nc.scalar.copy(out=hpJ[b], in_=hp_ps[:, 0:96])

                # ---------- attention mention-sum: 3 contiguous adds ----------
                ea_a = p1.tile([128, 4 * B, NE * NH], bft)
                eaT = p1.tile([128, 4 * B, NE * NH], bft)
                with nc.allow_low_precision("4-term mention-sum in bf16; feeds scale-invariant normalization"):
                    nc.vector.tensor_add(out=ea_a, in0=att_sb[:, :, 0, :], in1=att_sb[:, :, 1, :])
                    nc.vector.tensor_add(out=eaT, in0=att_sb[:, :, 2, :], in1=att_sb[:, :, 3, :])
                    nc.vector.tensor_add(out=eaT, in0=eaT, in1=ea_a)

                # ---------- per-chunk: products -> norm -> rsT -> extractors ----------
                hs_parts = []
                for ck in range(NCH1):
                    b, ih = ck // 2, ck % 2
                    # products for this chunk's 288 pairs (12 i x 24 j), 4 l-tiles
                    ht_raw = p1.tile([128, 4, CW], bft, tag="htraw", bufs=2, name=f"htraw{ck}")
                    with nc.allow_low_precision("bf16 pair products; normalization is scale-invariant"):
                        for lt in range(4):
                            blt = 4 * b + lt
                            prod = p1.tile([128, 12, NE, NH], bft, tag="prod", bufs=2)
                            in0 = (eaT[:, blt, :].rearrange("p (e h) -> p e h", h=NH)
                                   [:, 12 * ih: 12 * (ih + 1), :]
                                   .unsqueeze(2).broadcast_to([128, 12, NE, NH]))
                            in1 = (eaT[:, blt, :].rearrange("p (e h) -> p e h", h=NH)
                                   .unsqueeze(1).broadcast_to([128, 12, NE, NH]))
                            mul_eng = nc.gpsimd if ck == 3 else nc.vector
                            mul_eng.tensor_mul(out=prod, in0=in0, in1=in1)
                            nc.vector.tensor_reduce(
                                out=ht_raw[:, lt, :],
                                in_=prod[:].rearrange("p a b h -> p (a b) h"),
                                axis=X, op=ALU.add)
                        nc.vector.tensor_scalar_max(out=ht_raw[:], in0=ht_raw[:], scalar1=0.0)

                    # l-normalization: sum -> replicate -> 1/(x+eps) -> mul
                    sum_ps = ps1.tile([128, CW], f32, tag="mm288", bufs=3)
                    for lt in range(4):
                        nc.tensor.matmul(sum_ps[0:1, :], lhsT=ones_sb[:], rhs=ht_raw[:, lt, :],
                                         start=(lt == 0), stop=(lt == 3))
                    den_s = p1.tile([1, CW], f32, tag="dens", bufs=2)
                    nc.vector.tensor_scalar_add(out=den_s, in0=sum_ps[0:1, :], scalar1=1e-10)
                    den_r = p1.tile([1, CW], f32, tag="denr", bufs=2)
                    nc.vector.reciprocal_approx_fast(out=den_r, in_=den_s[:])
                    rep_ps = ps1.tile([128, CW], f32, tag="mm288", bufs=3)
                    nc.tensor.matmul(rep_ps, lhsT=onesrow_sb[:], rhs=den_r[:], start=True, stop=True)
                    rrep = p1.tile([128, CW], bft, tag="rrep", bufs=2)
                    nc.vector.tensor_copy(out=rrep, in_=rep_ps)
                    htn = p1.tile([128, 4, CW], bft, tag="htn", bufs=2, name=f"htn{ck}")
                    for lt in range(4):
                        nc.vector.tensor_mul(out=htn[:, lt, :], in0=ht_raw[:, lt, :], in1=rrep)

                    # rsT chunk [128, 6, 288]
                    rsT = p1.tile([128, 6, CW], bft, tag="rst", bufs=2, name=f"rst{ck}")
                    for ht in range(6):
                        rp = ps1.tile([128, CW], f32, tag="mm288", bufs=3)
                        for lt in range(4):
                            nc.tensor.matmul(rp, lhsT=seq_sb[:, 4 * b + lt, 128 * ht: 128 * (ht + 1)],
                                             rhs=htn[:, lt, :], start=(lt == 0), stop=(lt == 3))
                        nc.scalar.copy(out=rsT[:, ht, :], in_=rp)

                    # hs extractor chunk (indicator matmul folds hpart in)
                    hp3 = ps1.tile([128, CW], f32, tag="mm288", bufs=3)
                    for ht in range(6):
                        nc.tensor.matmul(hp3[0:96, :], lhsT=wh2_sb[:, ht, :], rhs=rsT[:, ht, :],
                                         start=(ht == 0), stop=False)
                    nc.tensor.matmul(hp3[0:96, :], lhsT=hpJ[b][:],
                                     rhs=I_sb[:, CW * ih: CW * (ih + 1)], start=False, stop=True)
                    hs_t = p1.tile([96, CW], bft, tag="hst", bufs=2)
                    nc.scalar.activation(out=hs_t, in_=hp3[0:96, :], func=AF.Tanh,
                                         bias=bh_sb[:, 0:1])
                    nc.sync.dma_start(out=hs_dr[:, CW * ck: CW * (ck + 1)], in_=hs_t[:])

                    # ts extractor chunk (indicator matmul folds tpart in)
                    for Et in range(6):
                        ep = ps1.tile([128, CW], f32, tag="mm288", bufs=3)
                        for ht in range(6):
                            nc.tensor.matmul(ep, lhsT=wt2_sb[:, ht, 128 * Et: 128 * (Et + 1)],
                                             rhs=rsT[:, ht, :], start=(ht == 0), stop=False)
                        nc.tensor.matmul(ep, lhsT=tpJ[b][:, 128 * Et: 128 * (Et + 1)],
                                         rhs=J_sb[:], start=False, stop=True)
                        nc.scalar.activation(out=ts_sb[:, Et, CW * ck: CW * (ck + 1)],
                                             in_=ep, func=AF.Tanh, bias=bt_sb[:, Et: Et + 1])

            # ---------- phase 2: bilinear + projection + classifier ----------
            with (
                tc.tile_pool(name="p2", bufs=1) as p2,
                tc.tile_pool(name="ps2", bufs=1, space="PSUM") as ps2,
            ):
                dma_engs = [nc.sync, nc.scalar, nc.gpsimd]
                for ck in range(NCH2):
                    sl2 = slice(CW2 * ck, CW2 * (ck + 1))
                    # bulk-prefetch the hs broadcast block for this chunk
                    b1c = p2.tile([128, NT2, CW2], bft, tag="b1c", bufs=2)
                    for kt2 in range(NT2):
                        s_l, a = kt2 // 6, kt2 % 6
                        r0 = (2 * a) * SL + s_l
                        r1 = (2 * a + 1) * SL + s_l
                        eng = dma_engs[kt2 % 3]
                        eng.dma_start(
                            out=b1c[0:64, kt2, :],
                            in_=bass.AP(tensor=hs_dr, offset=r0 * NP + CW2 * ck,
                                        ap=[[0, 64], [1, CW2]]))
                        eng.dma_start(
                            out=b1c[64:128, kt2, :],
                            in_=bass.AP(tensor=hs_dr, offset=r1 * NP + CW2 * ck,
                                        ap=[[0, 64], [1, CW2]]))
                    fps = []
                    for h in range(6):
                        fps.append(ps2.tile([128, CW2], f32, tag=f"feat{h}", bufs=1,
                                            name=f"fps{h}"))
                    for kt2 in range(NT2):
                        a = kt2 % 6
                        bl = p2.tile([128, CW2], bft, tag="bl", bufs=6)
                        mul_eng = nc.gpsimd if kt2 % 3 == 2 else nc.vector
                        mul_eng.tensor_mul(out=bl, in0=b1c[:, kt2, :], in1=ts_sb[:, a, sl2])
                        for h in range(6):
                            nc.tensor.matmul(fps[h], lhsT=wp_sb[:, kt2, 128 * h: 128 * (h + 1)],
                                             rhs=bl, start=(kt2 == 0), stop=(kt2 == NT2 - 1))
                    lgp = ps2.tile([C, CW2], f32, tag="lg", bufs=2)
                    for h in range(6):
                        fT = p2.tile([128, CW2], bft, tag="fT", bufs=3)
                        if h % 2 == 0:
                            nc.scalar.copy(out=fT, in_=fps[h])
                        else:
                            nc.vector.tensor_copy(out=fT, in_=fps[h])
                        nc.tensor.matmul(lgp, lhsT=wc_sb[:, h, :], rhs=fT,
                                         start=(h == 0), stop=(h == 5))
                    lgo = p2.tile([C, CW2], f32, tag="lgo", bufs=2)
                    nc.vector.tensor_copy(out=lgo, in_=lgp)
                    nc.sync.dma_start(out=out_lgT[:, sl2], in_=lgo[:])

    if not nc.is_finalized():
        nc.finalize()
    return nc


_NC_CACHE = None


def _host_finish(partials, bc):
    lgT = np.sum([np.asarray(p, dtype=np.float32) for p in partials], axis=0)
    logits = np.ascontiguousarray(lgT.T) + bc[None, :]
    mask = np.ones((NP, 1), np.float32)
    for b in range(B):
        for i in range(NE):
            mask[b * PPB + i * NE + i, 0] = 0.0
    return (logits * mask).astype(np.float32)


def kernel(**inputs):
    global _NC_CACHE
    from concourse.bass_utils import run_bass_kernel_spmd

    if _NC_CACHE is None:
        _NC_CACHE = build_bass()
    in_maps = _host_prep(inputs)
    res = run_bass_kernel_spmd(_NC_CACHE, in_maps, core_ids=list(range(NCORE)))
    kernel.last_results = res
    bc = np.asarray(inputs["bc"], dtype=np.float32)
    return _host_finish([res.results[c]["out_lgT"] for c in range(NCORE)], bc)


# revision 4
# speedup vs baseline: 1.5232x; 1.1322x over previous
"""Trainium2 Bass kernel v4 for nn_DocREModel_Triangle — collective-free,
phase-interleaved.

v4 over v3 (trace-driven):
  - full per-chunk pipeline: for each 288-pair chunk, products (DVE) ->
    normalization -> rsT -> extractors -> phase-2 feature+classifier are
    emitted together, so the PE works on chunk k's matmuls while the DVE
    computes chunk k+1's pair products (v3 ran the ~90us DVE product
    phase with the PE mostly idle, then a PE-bound phase 2).
  - gpsimd no longer issues DMAs (software-DGE dispatch was ~620ns per
    dma_start, 60us total) and no longer runs the big product muls
    (7.1us each vs 1.9us on DVE). It only takes 1/3 of the small
    phase-2 bl muls.
  - phase-2 chunk = 288 pairs (aligned with phase 1); b1 broadcast rows
    loaded just-in-time per tile (2 stride-0 DMAs, bufs=16, issued
    alternately on the two hardware DGE queues).
  - ts stored per-chunk (4 tiles) to avoid false tile-level WAR hazards
    between chunk k's phase-2 reads and chunk k+1's extractor writes.
  - wp loaded as 4 quarter-tiles so early phase-2 matmuls of chunk 0
    don't wait for the full 9.4 MB weight load.
  - PSUM: 6 feature accumulators + one shared [128,288] f32 tag (rp/ep/
    sums/replicate/classifier) with 2 bufs = 8 banks exactly.
"""

import numpy as np
import ml_dtypes

bf16 = ml_dtypes.bfloat16

B, L, H, NH = 2, 512, 768, 12
NE, NM = 24, 4
E, BS, C = 768, 64, 97
K = E // BS
NCORE = 8
SL = 64 // NCORE                 # 8 s-values per core
NP = B * NE * NE                 # 1152 pairs
PPB = NE * NE                    # 576 pairs per batch
CW = 288                         # pair chunk (12 i x 24 j)
NCH = NP // CW                   # 4
KST = K * SL * BS                # 6144 contraction rows per core
NT2 = KST // 128                 # 48 tiles, kt2 = s*6 + a (a = k//2)


def _host_prep(inputs):
    seq = np.ascontiguousarray(inputs["sequence_output"], dtype=np.float32)
    att = np.ascontiguousarray(inputs["attention"], dtype=np.float32)
    Wh = np.asarray(inputs["Wh"], dtype=np.float32)
    Wt = np.asarray(inputs["Wt"], dtype=np.float32)
    Wp = np.asarray(inputs["Wp"], dtype=np.float32)
    Wc = np.asarray(inputs["Wc"], dtype=np.float32)
    bh = np.asarray(inputs["bh"], dtype=np.float32)
    bt = np.asarray(inputs["bt"], dtype=np.float32)
    mpos = np.asarray(inputs["mention_pos"]).astype(np.int64)

    seq_bf = np.ascontiguousarray(seq.astype(bf16))
    wt1 = np.ascontiguousarray(Wt[:H].astype(bf16))
    wt2 = np.ascontiguousarray(Wt[H:].astype(bf16))
    bt_t = np.ascontiguousarray(bt.reshape(6, 128).T.astype(np.float32))
    wc_bf = np.ascontiguousarray(Wc.astype(bf16))
    wp4 = Wp.reshape(K, 64, BS, H)

    # mention rows (m, e): row = m*24 + e  -> [96, 2, 768] bf16
    ment = np.ascontiguousarray(
        np.stack([seq[b][mpos[b].T.reshape(-1)] for b in range(B)], 1)
        .astype(bf16))
    # attention gather, transposed, m-major: [b, l, (m e h)] bf16
    attT = np.zeros((B, L, NM * NE * NH), bf16)
    for b in range(B):
        G = att[b][:, mpos[b].reshape(-1), :]                 # [12, 96, 512]
        X = G.reshape(NH, NE, NM, L).transpose(3, 2, 1, 0)    # [512, 4m, 24e, 12h]
        attT[b] = X.reshape(L, -1).astype(bf16)
    attT = np.ascontiguousarray(attT)

    in_maps = []
    for c in range(NCORE):
        PERM_c = np.array([k * 64 + SL * c + sl for k in range(K) for sl in range(SL)])
        wh1p = np.ascontiguousarray(Wh[:H][:, PERM_c].astype(bf16))   # [768, 96]
        wh2p = np.ascontiguousarray(Wh[H:][:, PERM_c].astype(bf16))
        bh_c = np.ascontiguousarray(bh[PERM_c].reshape(96, 1).astype(np.float32))
        wp_c = np.ascontiguousarray(
            wp4[:, SL * c: SL * (c + 1)].transpose(1, 0, 2, 3)
            .reshape(KST, H).astype(bf16))
        in_maps.append({
            "seq_bf": seq_bf,
            "att_T": attT,
            "ment": ment,
            "wh1p": wh1p,
            "wh2p": wh2p,
            "wt1": wt1,
            "wt2": wt2,
            "bh_c": bh_c,
            "bt_t": bt_t,
            "wp_sl": wp_c,
            "wc_bf": wc_bf,
        })
    return in_maps


def _build_consts():
    S2 = np.zeros((96, NE), dtype=bf16)
    for m in range(NM):
        for e in range(NE):
            S2[m * NE + e, e] = 1.0
    ones_bf = np.ones((128, 1), dtype=bf16)
    ones_row = np.ones((1, 128), dtype=np.float32)
    Jind = np.zeros((NE, CW), dtype=bf16)
    for il in range(12):
        for j in range(NE):
            Jind[j, il * NE + j] = 1.0
    Iind = np.zeros((NE, 2 * CW), dtype=bf16)
    for ih in range(2):
        for il in range(12):
            for j in range(NE):
                Iind[ih * 12 + il, ih * CW + il * NE + j] = 1.0
    return S2, ones_bf, ones_row, Jind, Iind


def build_bass():
    import concourse.bass as bass
    import concourse.mybir as mybir
    import concourse.tile as tile
    from concourse.bacc import Bacc

    f32 = mybir.dt.float32
    bft = mybir.dt.bfloat16
    AF = mybir.ActivationFunctionType
    ALU = mybir.AluOpType
    X = mybir.AxisListType.X

    nc = Bacc("TRN2", num_devices=NCORE)

    seq_bf = nc.dram_tensor("seq_bf", [B, L, H], bft, kind="ExternalInput")
    att_T = nc.dram_tensor("att_T", [B, L, NM * NE * NH], bft, kind="ExternalInput")
    ment = nc.dram_tensor("ment", [96, B, H], bft, kind="ExternalInput")
    wh1p = nc.dram_tensor("wh1p", [H, 96], bft, kind="ExternalInput")
    wh2p = nc.dram_tensor("wh2p", [H, 96], bft, kind="ExternalInput")
    wt1 = nc.dram_tensor("wt1", [H, E], bft, kind="ExternalInput")
    wt2 = nc.dram_tensor("wt2", [H, E], bft, kind="ExternalInput")
    bh_c = nc.dram_tensor("bh_c", [96, 1], f32, kind="ExternalInput")
    bt_t = nc.dram_tensor("bt_t", [128, 6], f32, kind="ExternalInput")
    wp_sl = nc.dram_tensor("wp_sl", [KST, H], bft, kind="ExternalInput")
    wc_bf = nc.dram_tensor("wc_bf", [H, C], bft, kind="ExternalInput")
    out_lgT = nc.dram_tensor("out_lgT", [C, NP], f32, kind="ExternalOutput")

    hs_dr = nc.dram_tensor("hs_dr", [96, NP], bft)

    S2_np, ones_np, onesrow_np, J_np, I_np = _build_consts()
    S2_dr = nc.inline_tensor(S2_np, "s2_const")
    ones_dr = nc.inline_tensor(ones_np, "ones_const")
    onesrow_dr = nc.inline_tensor(onesrow_np, "onesrow_const")
    J_dr = nc.inline_tensor(J_np, "j_const")
    I_dr = nc.inline_tensor(I_np, "i_const")

    with tile.TileContext(nc) as tc:
        with tc.tile_pool(name="gpool", bufs=1) as gpool:
            # ---------- persistent ----------
            wp_q = [gpool.tile([128, 12, H], bft, name=f"wpq{q}") for q in range(4)]
            wc_sb = gpool.tile([128, 6, C], bft)
            ts_ck = [gpool.tile([128, 6, CW], bft, name=f"tsck{ck}") for ck in range(NCH)]
            tpJ = [gpool.tile([NE, E], bft, name=f"tpJ{b}") for b in range(B)]
            hpJ = [gpool.tile([NE, 96], bft, name=f"hpJ{b}") for b in range(B)]
            eaT = gpool.tile([128, 4 * B, NE * NH], bft)
            bt_sb = gpool.tile([128, 6], f32)
            bh_sb = gpool.tile([96, 1], f32)
            ones_sb = gpool.tile([128, 1], bft)
            onesrow_sb = gpool.tile([1, 128], f32)
            J_sb = gpool.tile([NE, CW], bft)
            I_sb = gpool.tile([NE, 2 * CW], bft)
            nc.gpsimd.dma_start(out=bt_sb, in_=bt_t[:])
            nc.gpsimd.dma_start(out=bh_sb, in_=bh_c[:])
            nc.gpsimd.dma_start(out=ones_sb, in_=ones_dr[:])
            nc.gpsimd.dma_start(out=onesrow_sb, in_=onesrow_dr[:])
            nc.gpsimd.dma_start(out=J_sb, in_=J_dr[:])
            nc.gpsimd.dma_start(out=I_sb, in_=I_dr[:])

            with (
                tc.tile_pool(name="p1", bufs=1) as p1,
                tc.tile_pool(name="ps1", bufs=1, space="PSUM") as ps1,
            ):
                # ---------- main-phase inputs ----------
                seq_sb = p1.tile([128, 4 * B, H], bft)
                wh2_sb = p1.tile([128, 6, 96], bft)
                wt2_sb = p1.tile([128, 6, E], bft)

                with (
                    tc.tile_pool(name="p0", bufs=1) as p0,
                    tc.tile_pool(name="ps0", bufs=1, space="PSUM") as ps0,
                ):
                    # ---------- early inputs, attention first ----------
                    att_sb = p0.tile([128, 4 * B, NM, NE * NH], bft)
                    for b in range(B):
                        nc.sync.dma_start(
                            out=att_sb[:, 4 * b: 4 * b + 4, :, :],
                            in_=att_T[b].rearrange("(a p) (m x) -> p a m x", p=128, m=NM))
                    ment_sb = p0.tile([96, B, H], bft)
                    nc.scalar.dma_start(out=ment_sb, in_=ment[:])
                    S2_sb = p0.tile([96, NE], bft)
                    nc.scalar.dma_start(out=S2_sb, in_=S2_dr[:])
                    wh1_sb = p0.tile([128, 6, 96], bft)
                    nc.scalar.dma_start(out=wh1_sb, in_=wh1p[:].rearrange("(a p) e -> p a e", p=128))
                    wt1_sb = p0.tile([128, 6, E], bft)
                    nc.scalar.dma_start(out=wt1_sb, in_=wt1[:].rearrange("(a p) e -> p a e", p=128))
                    for b in range(B):
                        nc.sync.dma_start(
                            out=seq_sb[:, 4 * b: 4 * b + 4, :],
                            in_=seq_bf[b].rearrange("(a p) h -> p a h", p=128))
                    nc.scalar.dma_start(out=wh2_sb, in_=wh2p[:].rearrange("(a p) e -> p a e", p=128))
                    nc.scalar.dma_start(out=wt2_sb, in_=wt2[:].rearrange("(a p) e -> p a e", p=128))
                    # phase-2 weights stream in the background, quarter by quarter
                    for q in range(4):
                        nc.sync.dma_start(
                            out=wp_q[q],
                            in_=wp_sl[12 * 128 * q: 12 * 128 * (q + 1), :]
                            .rearrange("(a p) h -> p a h", p=128))
                    nc.scalar.dma_start(out=wc_sb, in_=wc_bf[:].rearrange("(a p) c -> p a c", p=128))

                    # ---------- mention-sum (batch 0 first) ----------
                    ea_a = p0.tile([128, 4 * B, NE * NH], bft)
                    with nc.allow_low_precision("4-term mention-sum in bf16"):
                        for b in range(B):
                            sb = slice(4 * b, 4 * b + 4)
                            nc.vector.tensor_add(out=ea_a[:, sb, :], in0=att_sb[:, sb, 0, :],
                                                 in1=att_sb[:, sb, 1, :])
                            nc.vector.tensor_add(out=eaT[:, sb, :], in0=att_sb[:, sb, 2, :],
                                                 in1=att_sb[:, sb, 3, :])
                            nc.vector.tensor_add(out=eaT[:, sb, :], in0=eaT[:, sb, :],
                                                 in1=ea_a[:, sb, :])

                    # ---------- entity pooling + tpJ/hpJ ----------
                    exp_sb = p0.tile([96, B, H], bft)
                    for b in range(B):
                        nc.scalar.activation(out=exp_sb[:, b, :], in_=ment_sb[:, b, :], func=AF.Exp)
                    eeT = p0.tile([128, 6, B * NE], bft)
                    for ht in range(6):
                        for b in range(B):
                            tr = ps0.tile([128, B * NE], f32, tag="mm48", bufs=2)
                            nc.tensor.matmul(tr[:, 0:NE], lhsT=exp_sb[:, b, 128 * ht: 128 * (ht + 1)],
                                             rhs=S2_sb[:], start=True, stop=True)
                            nc.scalar.activation(out=eeT[:, ht, NE * b: NE * (b + 1)],
                                                 in_=tr[:, 0:NE], func=AF.Ln)
                    for b in range(B):
                        for half in range(2):
                            tp_ps = ps0.tile([NE, 384], f32, tag="tpj", bufs=2)
                            for ht in range(6):
                                nc.tensor.matmul(tp_ps, lhsT=eeT[:, ht, NE * b: NE * (b + 1)],
                                                 rhs=wt1_sb[:, ht, 384 * half: 384 * (half + 1)],
                                                 start=(ht == 0), stop=(ht == 5))
                            nc.scalar.copy(out=tpJ[b][:, 384 * half: 384 * (half + 1)], in_=tp_ps)
                        hp_ps = ps0.tile([NE, 384], f32, tag="tpj", bufs=2)
                        for ht in range(6):
                            nc.tensor.matmul(hp_ps[:, 0:96], lhsT=eeT[:, ht, NE * b: NE * (b + 1)],
                                             rhs=wh1_sb[:, ht, :], start=(ht == 0), stop=(ht == 5))
                        nc.scalar.copy(out=hpJ[b], in_=hp_ps[:, 0:96])
                # ---- p0/ps0 closed: att/ment/exp/wh1/wt1/eeT freed ----

                with (
                    tc.tile_pool(name="p2", bufs=1) as p2,
                    tc.tile_pool(name="ps2", bufs=1, space="PSUM") as ps2,
                ):
                    for ck in range(NCH):
                        b, ih = ck // 2, ck % 2
                        # ---------- pair products (DVE) ----------
                        ht_raw = p1.tile([128, 4, CW], bft, tag="htraw", bufs=2, name=f"htraw{ck}")
                        with nc.allow_low_precision("bf16 pair products; scale-invariant"):
                            for lt in range(4):
                                blt = 4 * b + lt
                                prod = p1.tile([128, 12, NE, NH], bft, tag="prod", bufs=2)
                                in0 = (eaT[:, blt, :].rearrange("p (e h) -> p e h", h=NH)
                                       [:, 12 * ih: 12 * (ih + 1), :]
                                       .unsqueeze(2).broadcast_to([128, 12, NE, NH]))
                                in1 = (eaT[:, blt, :].rearrange("p (e h) -> p e h", h=NH)
                                       .unsqueeze(1).broadcast_to([128, 12, NE, NH]))
                                nc.vector.tensor_mul(out=prod, in0=in0, in1=in1)
                                nc.vector.tensor_reduce(
                                    out=ht_raw[:, lt, :],
                                    in_=prod[:].rearrange("p a b h -> p (a b) h"),
                                    axis=X, op=ALU.add)
                            nc.vector.tensor_scalar_max(out=ht_raw[:], in0=ht_raw[:], scalar1=0.0)

                        # ---------- l-normalization ----------
                        sum_ps = ps2.tile([128, CW], f32, tag="big", bufs=2)
                        for lt in range(4):
                            nc.tensor.matmul(sum_ps[0:1, :], lhsT=ones_sb[:], rhs=ht_raw[:, lt, :],
                                             start=(lt == 0), stop=(lt == 3))
                        den_s = p1.tile([1, CW], f32, tag="dens", bufs=2)
                        nc.vector.tensor_scalar_add(out=den_s, in0=sum_ps[0:1, :], scalar1=1e-10)
                        den_r = p1.tile([1, CW], f32, tag="denr", bufs=2)
                        nc.vector.reciprocal_approx_fast(out=den_r, in_=den_s[:])
                        rep_ps = ps2.tile([128, CW], f32, tag="big", bufs=2)
                        nc.tensor.matmul(rep_ps, lhsT=onesrow_sb[:], rhs=den_r[:], start=True, stop=True)
                        rrep = p1.tile([128, CW], bft, tag="rrep", bufs=2)
                        nc.vector.tensor_copy(out=rrep, in_=rep_ps)
                        htn = p1.tile([128, 4, CW], bft, tag="htn", bufs=2, name=f"htn{ck}")
                        for lt in range(4):
                            nc.vector.tensor_mul(out=htn[:, lt, :], in0=ht_raw[:, lt, :], in1=rrep)

                        # ---------- rsT ----------
                        rsT = p1.tile([128, 6, CW], bft, tag="rst", bufs=2, name=f"rst{ck}")
                        for ht in range(6):
                            rp = ps2.tile([128, CW], f32, tag="big", bufs=2)
                            for lt in range(4):
                                nc.tensor.matmul(rp, lhsT=seq_sb[:, 4 * b + lt, 128 * ht: 128 * (ht + 1)],
                                                 rhs=htn[:, lt, :], start=(lt == 0), stop=(lt == 3))
                            nc.scalar.copy(out=rsT[:, ht, :], in_=rp)

                        # ---------- hs extractor -> hs_dr ----------
                        hp3 = ps2.tile([128, CW], f32, tag="big", bufs=2)
                        for ht in range(6):
                            nc.tensor.matmul(hp3[0:96, :], lhsT=wh2_sb[:, ht, :], rhs=rsT[:, ht, :],
                                             start=(ht == 0), stop=False)
                        nc.tensor.matmul(hp3[0:96, :], lhsT=hpJ[b][:],
                                         rhs=I_sb[:, CW * ih: CW * (ih + 1)], start=False, stop=True)
                        hs_t = p1.tile([96, CW], bft, tag="hst", bufs=2)
                        nc.scalar.activation(out=hs_t, in_=hp3[0:96, :], func=AF.Tanh,
                                             bias=bh_sb[:, 0:1])
                        nc.sync.dma_start(out=hs_dr[:, CW * ck: CW * (ck + 1)], in_=hs_t[:])

                        # ---------- ts extractor ----------
                        for Et in range(6):
                            ep = ps2.tile([128, CW], f32, tag="big", bufs=2)
                            for ht in range(6):
                                nc.tensor.matmul(ep, lhsT=wt2_sb[:, ht, 128 * Et: 128 * (Et + 1)],
                                                 rhs=rsT[:, ht, :], start=(ht == 0), stop=False)
                            nc.tensor.matmul(ep, lhsT=tpJ[b][:, 128 * Et: 128 * (Et + 1)],
                                             rhs=J_sb[:], start=False, stop=True)
                            nc.scalar.activation(out=ts_ck[ck][:, Et, :],
                                                 in_=ep, func=AF.Tanh, bias=bt_sb[:, Et: Et + 1])

                        # ---------- phase 2 for this chunk ----------
                        fps = []
                        for h in range(6):
                            fps.append(ps2.tile([128, CW], f32, tag=f"feat{h}", bufs=1,
                                                name=f"fps{h}"))
                        for kt2 in range(NT2):
                            s_l, a = kt2 // 6, kt2 % 6
                            r0 = (2 * a) * SL + s_l
                            r1 = (2 * a + 1) * SL + s_l
                            b1r = p2.tile([128, CW], bft, tag="b1r", bufs=16)
                            eng = nc.sync if kt2 % 2 == 0 else nc.scalar
                            eng.dma_start(
                                out=b1r[0:64, :],
                                in_=bass.AP(tensor=hs_dr, offset=r0 * NP + CW * ck,
                                            ap=[[0, 64], [1, CW]]))
                            eng.dma_start(
                                out=b1r[64:128, :],
                                in_=bass.AP(tensor=hs_dr, offset=r1 * NP + CW * ck,
                                            ap=[[0, 64], [1, CW]]))
                            bl = p2.tile([128, CW], bft, tag="bl", bufs=8)
                            mul_eng = nc.gpsimd if kt2 % 3 == 2 else nc.vector
                            mul_eng.tensor_mul(out=bl, in0=b1r, in1=ts_ck[ck][:, a, :])
                            for h in range(6):
                                nc.tensor.matmul(fps[h], lhsT=wp_q[kt2 // 12][:, kt2 % 12, 128 * h: 128 * (h + 1)],
                                                 rhs=bl, start=(kt2 == 0), stop=(kt2 == NT2 - 1))
                        lgp = ps2.tile([128, CW], f32, tag="big", bufs=2)
                        for h in range(6):
                            fT = p2.tile([128, CW], bft, tag="fT", bufs=3)
                            if h % 2 == 0:
                                nc.scalar.copy(out=fT, in_=fps[h])
                            else:
                                nc.vector.tensor_copy(out=fT, in_=fps[h])
                            nc.tensor.matmul(lgp[0:C, :], lhsT=wc_sb[:, h, :], rhs=fT,
                                             start=(h == 0), stop=(h == 5))
                        lgo = p2.tile([C, CW], f32, tag="lgo", bufs=2)
                        nc.vector.tensor_copy(out=lgo, in_=lgp[0:C, :])
                        nc.sync.dma_start(out=out_lgT[:, CW * ck: CW * (ck + 1)], in_=lgo[:])

    if not nc.is_finalized():
        nc.finalize()
    return nc


_NC_CACHE = None


def _host_finish(partials, bc):
    lgT = np.sum([np.asarray(p, dtype=np.float32) for p in partials], axis=0)
    logits = np.ascontiguousarray(lgT.T) + bc[None, :]
    mask = np.ones((NP, 1), np.float32)
    for b in range(B):
        for i in range(NE):
            mask[b * PPB + i * NE + i, 0] = 0.0
    return (logits * mask).astype(np.float32)


def kernel(**inputs):
    global _NC_CACHE
    from concourse.bass_utils import run_bass_kernel_spmd

    if _NC_CACHE is None:
        _NC_CACHE = build_bass()
    in_maps = _host_prep(inputs)
    res = run_bass_kernel_spmd(_NC_CACHE, in_maps, core_ids=list(range(NCORE)))
    kernel.last_results = res
    bc = np.asarray(inputs["bc"], dtype=np.float32)
    return _host_finish([res.results[c]["out_lgT"] for c in range(NCORE)], bc)
